# revision 1
# baseline (speedup 1.0000x reference)
"""Trainium2 Bass kernel for the Baller2Vec-style transformer encoder.

Strategy: pure data parallel over batch (B=128) across 8 NeuronCores
(16 batch elements per core, processed as 8 pairs of 2).  Activations are
kept feature-major on-chip ([d_model on partitions, tokens on free]) so
every linear layer needs no activation transposes; V is produced
token-major directly from the same matmul family so attention needs no
transposes either.  Matmuls run in bf16 with fp32 PSUM accumulation;
softmax normalization, LayerNorm statistics and the residual stream stay
fp32.  Cross-partition reductions (LN stats, softmax row sums) and
partition broadcasts (mean/rstd/recip rows) are done on the PE with
ones/selector rank-1 matmuls — fp32 for the precision-critical
broadcasts.
"""
import os
import sys
import numpy as np

sys.path.insert(0, '/opt/trn_rl_repo')

import ml_dtypes
import concourse.bass as bass
import concourse.mybir as mybir
from concourse import tile
from concourse.bass_utils import run_bass_kernel_spmd
from concourse.vector_clock import ScopedClock

# ---------------------------------------------------------------- constants
P, T, E, D, H, F, L, V, C = 10, 20, 64, 512, 8, 2048, 6, 512, 9
B = 128
S = (P + 2) * T          # 240
HD = D // H              # 64
SCALE = float(np.sqrt(D))
HSCALE = 1.0 / float(np.sqrt(HD))
NCORES = 8
BPC = B // NCORES        # 16 batch elements per core
NPAIR = BPC // 2         # 8 pairs
PC = 2 * S               # 480 columns per pair
NC_CH = D // 128         # 4 feature chunks
NF_CH = F // 128         # 16 ffn chunks
TK = 120                 # tk tile: 240 = 2 x 120
EPS = 1e-5
NEG = -30000.0

bf16 = mybir.dt.bfloat16
f32 = mybir.dt.float32
AF = mybir.ActivationFunctionType
ALU = mybir.AluOpType


def _mask_np():
    m = np.full((S, S), -np.inf, dtype=np.float32)
    bs, cs = P * T, P * T + T
    for s in range(T):
        a, b_ = P * s, P * s + P
        m[a:b_, :b_] = 0.0
        m[a:b_, bs:bs + s + 1] = 0.0
        m[a:b_, cs:cs + s + 1] = 0.0
        for r in (bs + s, cs + s):
            m[r, :b_] = 0.0
            m[r, bs:bs + s + 1] = 0.0
            m[r, cs:cs + s + 1] = 0.0
    return m


# ---------------------------------------------------------------- tile patch
MAX_WAITS_PER_INST = 1


def _patched_drain_and_barrier(self, tick_clock, wait_clock):
    nc = self.nc
    probe = nc.sync.nop(nofuse=True)
    wait_clock.add_sem_waits(probe.ins, ScopedClock({None: tick_clock.global_clock}))
    si = probe.ins.sync_info
    waits = list(si.on_wait) if si is not None else []
    K = MAX_WAITS_PER_INST
    probe.ins.sync_info = mybir.SyncInfo(on_wait=waits[:K], on_update=[])
    for j in range(K, len(waits), K):
        w = nc.sync.nop(nofuse=True)
        w.ins.sync_info = mybir.SyncInfo(on_wait=waits[j:j + K], on_update=[])
    nc.sync.drain()
    nc.all_engine_barrier()
    popped = nc._tile_sem_poison_stack.pop()
    assert popped is self._sem_poison
    nc.clear_and_free_semaphores(list(self.sems.allocated().values()))
    nc.all_engine_barrier()


tile.TileContext._drain_and_barrier = _patched_drain_and_barrier


# ---------------------------------------------------------------- builder
def build(n_layers=L, n_pairs=NPAIR):
    nc = bass.Bass("TRN2", target_bir_lowering=False)
    dt_in = {
        'pinT': ([E + 2, BPC * T * P], bf16),
        'ballT': ([E + 2, BPC * T], bf16),
        'maskb': ([TK, 2, S], bf16),          # maskbit^T tiles
        'pw1': ([E + 2, 128], bf16),
        'pw2': ([128, NC_CH * 128], bf16),
        'bw1': ([E + 2, 128], bf16),
        'bw2': ([128, NC_CH * 128], bf16),
        'pb1': ([128, 1], f32),
        'pb2s': ([128, NC_CH], f32),
        'bb1': ([128, 1], f32),
        'bb2s': ([128, NC_CH], f32),
        'wqkv': ([L, 128, NC_CH, 3 * D], bf16),
        'wo': ([L, 128, NC_CH, D], bf16),
        'wf1': ([L, 128, NC_CH, F], bf16),
        'wf2': ([L, 128, NF_CH, D], bf16),
        'cvec': ([128, L, 48], f32),
        'selcf': ([96, 3, NC_CH * 128], bf16),
        'onesbf': ([128, 1], bf16),
        'onesf': ([1, 128], f32),
        'clsw': ([128, NC_CH, C], f32),
        'clsb': ([C, 1], f32),
        'clse': ([128, NC_CH], f32),
    }
    dins = {k: nc.dram_tensor(k, sh, dt, kind="ExternalInput")
            for k, (sh, dt) in dt_in.items()}
    dout = nc.dram_tensor("out", [C, BPC], f32, kind="ExternalOutput")

    with tile.TileContext(nc) as tc:
        _body(nc, tc, dins, dout, n_layers, n_pairs)
    _split_waits(nc)
    return nc


def _split_waits(nc, K=1):
    """walrus codegen allows only one sync-wait per instruction here; hoist
    extra waits onto same-engine NoOps inserted just before."""
    wsid = [0]
    for bb in nc.m.functions[0].blocks:
        out = []
        changed = False
        for inst in bb.instructions:
            si = inst.sync_info
            waits = list(si.on_wait) if si is not None else []
            if len(waits) > K:
                changed = True
                extra = waits[:-K]
                for j in range(0, len(extra), K):
                    nop = mybir.InstNoOp(name=f"WSNOP-{wsid[0]}")
                    wsid[0] += 1
                    nop.engine = inst.engine
                    nop.sync_info = mybir.SyncInfo(on_wait=extra[j:j + K],
                                                   on_update=[])
                    out.append(nop)
                inst.sync_info = mybir.SyncInfo(on_wait=waits[-K:],
                                                on_update=list(si.on_update))
            out.append(inst)
        if changed:
            bb.instructions = out


def _body(nc, tc, dins, dout, n_layers, n_pairs):
    import contextlib
    ctx = contextlib.ExitStack()
    with ctx:
        persist = ctx.enter_context(tc.tile_pool(name="persist", bufs=1))
        # persistent activations, feature-major fp32
        X = persist.tile([128, NC_CH, BPC * S], f32)

        # constants
        maskb = persist.tile([TK, 2, S], bf16)
        nc.sync.dma_start(maskb[:], dins['maskb'][:])
        cvec = persist.tile([128, L, 48], f32)
        nc.sync.dma_start(cvec[:], dins['cvec'][:])
        selcf = persist.tile([96, 3, NC_CH * 128], bf16)
        nc.sync.dma_start(selcf[:], dins['selcf'][:])
        onesbf = persist.tile([128, 1], bf16)
        nc.sync.dma_start(onesbf[:], dins['onesbf'][:])
        onesf = persist.tile([1, 128], f32)
        nc.sync.dma_start(onesf[:], dins['onesf'][:])
        onesrbf = persist.tile([1, 128], bf16)
        nc.vector.tensor_copy(onesrbf[:], onesf[:])
        clsw = persist.tile([128, NC_CH, C], f32)
        nc.sync.dma_start(clsw[:], dins['clsw'][:])
        clsb = persist.tile([C, 1], f32)
        nc.sync.dma_start(clsb[:], dins['clsb'][:])
        clse = persist.tile([128, NC_CH], f32)
        nc.sync.dma_start(clse[:], dins['clse'][:])

        # -------------------------------------------------- front end
        with tc.tile_pool(name="fe", bufs=1) as fe, \
             tc.tile_pool(name="feps", bufs=2, space="PSUM") as feps:
            pint = fe.tile([E + 2, BPC * T * P], bf16)
            nc.sync.dma_start(pint[:], dins['pinT'][:])
            ballT = fe.tile([E + 2, BPC * T], bf16)
            nc.sync.dma_start(ballT[:], dins['ballT'][:])
            pw1 = fe.tile([E + 2, 128], bf16)
            nc.sync.dma_start(pw1[:], dins['pw1'][:])
            pw2 = fe.tile([128, NC_CH * 128], bf16)
            nc.sync.dma_start(pw2[:], dins['pw2'][:])
            bw1 = fe.tile([E + 2, 128], bf16)
            nc.sync.dma_start(bw1[:], dins['bw1'][:])
            bw2 = fe.tile([128, NC_CH * 128], bf16)
            nc.sync.dma_start(bw2[:], dins['bw2'][:])
            pb1 = fe.tile([128, 1], f32)
            nc.sync.dma_start(pb1[:], dins['pb1'][:])
            pb2s = fe.tile([128, NC_CH], f32)
            nc.sync.dma_start(pb2s[:], dins['pb2s'][:])
            bb1 = fe.tile([128, 1], f32)
            nc.sync.dma_start(bb1[:], dins['bb1'][:])
            bb2s = fe.tile([128, NC_CH], f32)
            nc.sync.dma_start(bb2s[:], dins['bb2s'][:])

            # player tokens: 8 blocks of 400 (= 2b x 200)
            for pi in range(NPAIR):
                h1p = feps.tile([128, 400], f32, tag="feps")
                nc.tensor.matmul(h1p[:], pw1[:], pint[:, pi * 400:(pi + 1) * 400],
                                 start=True, stop=True)
                h1b = fe.tile([128, 400], bf16, tag="h1b")
                nc.vector.tensor_scalar(h1b[:], h1p[:], pb1[:, 0:1], 0.0,
                                        ALU.add, ALU.max)
                for c in range(NC_CH):
                    pfp = feps.tile([128, 400], f32, tag="feps")
                    nc.tensor.matmul(pfp[:], pw2[:, c * 128:(c + 1) * 128], h1b[:],
                                     start=True, stop=True)
                    # scatter into X: col = b*240 + t*12 + p = 12*(b*20+t) + p
                    Xc = X[:, c, :].rearrange("p (j k) -> p j k",
                                              j=BPC * T, k=P + 2)
                    dst = Xc[:, 2 * pi * T:(2 * pi + 2) * T, 0:P]
                    nc.scalar.activation(dst, pfp[:].rearrange(
                        "p (j k) -> p j k", j=2 * T, k=P),
                        AF.Identity, bias=pb2s[:, c:c + 1], scale=SCALE)
            # ball tokens: all 16 b at once (320 cols)
            h1bl = feps.tile([128, 320], f32, tag="feps")
            nc.tensor.matmul(h1bl[:], bw1[:], ballT[:], start=True, stop=True)
            h1blb = fe.tile([128, 320], bf16, tag="h1b")
            nc.vector.tensor_scalar(h1blb[:], h1bl[:], bb1[:, 0:1], 0.0,
                                    ALU.add, ALU.max)
            for c in range(NC_CH):
                bfp = feps.tile([128, 320], f32, tag="feps")
                nc.tensor.matmul(bfp[:], bw2[:, c * 128:(c + 1) * 128], h1blb[:],
                                 start=True, stop=True)
                Xc = X[:, c, :].rearrange("p (j k) -> p j k", j=BPC * T, k=P + 2)
                src = bfp[:].rearrange("p (j k) -> p j k", j=BPC * T, k=1)
                nc.scalar.activation(Xc[:, :, P:P + 1], src,
                                     AF.Identity, bias=bb2s[:, c:c + 1], scale=SCALE)
                # cls token: 0*in + cls_e
                nc.scalar.activation(Xc[:, :, P + 1:P + 2], src,
                                     AF.Identity, bias=clse[:, c:c + 1], scale=0.0)

        # -------------------------------------------------- layer loop
        wpool = ctx.enter_context(tc.tile_pool(name="wq", bufs=2))
        wpool1 = ctx.enter_context(tc.tile_pool(name="wf", bufs=1))
        spool = ctx.enter_context(tc.tile_pool(name="scr", bufs=1))
        spool2 = ctx.enter_context(tc.tile_pool(name="scr2", bufs=2))
        tpool = ctx.enter_context(tc.tile_pool(name="tmp", bufs=2))
        mm_ps = ctx.enter_context(tc.tile_pool(name="mmps", bufs=2, space="PSUM"))
        sc_ps = ctx.enter_context(tc.tile_pool(name="scps", bufs=2, space="PSUM"))
        o_ps = ctx.enter_context(tc.tile_pool(name="ops", bufs=2, space="PSUM"))
        rs_ps = ctx.enter_context(tc.tile_pool(name="rsps", bufs=2, space="PSUM"))

        for l in range(n_layers):
            wq = wpool1.tile([128, NC_CH, 3 * D], bf16, tag="wq")
            nc.sync.dma_start(wq[:], dins['wqkv'][l])
            wo = wpool.tile([128, NC_CH, D], bf16, tag="wo")
            nc.sync.dma_start(wo[:], dins['wo'][l])
            wf1 = wpool1.tile([128, NC_CH, F], bf16, tag="wf1")
            nc.sync.dma_start(wf1[:], dins['wf1'][l])
            wf2 = wpool1.tile([128, NF_CH, D], bf16, tag="wf2")
            nc.sync.dma_start(wf2[:], dins['wf2'][l])
            cv = cvec[:, l, :]

            for pi in range(n_pairs):
                pc = slice(pi * PC, (pi + 1) * PC)
                # ---------------- cast x pair to bf16
                xb = spool.tile([128, NC_CH, PC], bf16, tag="xb")
                for c in range(NC_CH):
                    nc.vector.tensor_copy(xb[:, c, :], X[:, c, pc])
                # ---------------- q', k  (feature-major bf16)
                qkb = spool.tile([128, 8, PC], bf16, tag="mid8")
                for j in range(8):
                    ps = mm_ps.tile([128, PC], f32, tag="mm")
                    for c in range(NC_CH):
                        nc.tensor.matmul(ps[:], wq[:, c, j * 128:(j + 1) * 128],
                                         xb[:, c, :],
                                         start=(c == 0), stop=(c == NC_CH - 1))
                    scl = HSCALE if j < 4 else 1.0
                    nc.scalar.activation(qkb[:, j, :], ps[:], AF.Identity,
                                         bias=cv[:, j:j + 1], scale=scl)
                # ---------------- v (token-major bf16)
                vtm = spool.tile([128, 4, D], bf16, tag="vtm")
                for s4 in range(4):      # (b, half): 120 tokens each
                    b, hf = divmod(s4, 2)
                    lo = b * S + hf * TK
                    ps = mm_ps.tile([128, D], f32, tag="mm")
                    for c in range(NC_CH):
                        nc.tensor.matmul(ps[:TK, :], xb[:, c, lo:lo + TK],
                                         wq[:, c, 2 * D:3 * D],
                                         start=(c == 0), stop=(c == NC_CH - 1))
                    nc.scalar.activation(vtm[:TK, s4, :], ps[:TK, :], AF.Identity)
                # ---------------- scores -> exp -> mask
                eT = spool2.tile([128, 32, S], bf16, tag="big16")
                for h in range(H):
                    hb = (h % 2) * 64
                    jq, jk = h // 2, 4 + h // 2
                    for b in range(2):
                        seg = (h * 2 + b) * 2
                        ps = sc_ps.tile([TK, 2, S], f32, tag="sc")
                        for s in range(2):
                            nc.tensor.matmul(
                                ps[:, s, :],
                                qkb[hb:hb + 64, jk, b * S + s * TK:b * S + s * TK + TK],
                                qkb[hb:hb + 64, jq, b * S:(b + 1) * S],
                                start=True, stop=True)
                        tmp = tpool.tile([TK, 2, S], f32, tag="exp")
                        nc.scalar.activation(tmp[:], ps[:], AF.Exp)
                        nc.vector.tensor_tensor(eT[:TK, seg:seg + 2, :], tmp[:],
                                                maskb[:, :, :], ALU.mult)
                # ---------------- row sums (psum base must be 0/32/64)
                rsts = [rs_ps.tile([96, PC], f32, tag="rs", name=f"rs{i}")
                        for i in range(3)]
                for h in range(H):
                    ti, sub = divmod(h, 3)
                    base = sub * 32
                    eTh = eT[:TK, :, :].rearrange("p (hb s) n -> p s hb n", s=2)
                    for s in range(2):
                        nc.tensor.matmul(
                            rsts[ti][base:base + 1, :],
                            onesbf[:TK, 0:1],
                            eTh[:, s, 2 * h:2 * h + 2, :],
                            start=(s == 0), stop=(s == 1))
                stg = spool.tile([96, 3, PC], bf16, tag="stg")
                nc.gpsimd.memset(stg[:], 0.0)
                with nc.allow_low_precision(reason="softmax recip broadcast"):
                    for h in range(H):
                        ti, sub = divmod(h, 3)
                        base = sub * 32
                        nc.vector.reciprocal(stg[base:base + 1, ti, :],
                                             rsts[ti][base:base + 1, :])
                # ---------------- PV + normalize (fp32 selector broadcast)
                ofm = spool.tile([128, NC_CH, PC], bf16, tag="ofm")
                for c in range(NC_CH):
                    bc = sc_ps.tile([128, PC], f32, tag="sc")
                    for ti in range(3):
                        nc.tensor.matmul(bc[:],
                                         selcf[:, ti, c * 128:(c + 1) * 128],
                                         stg[:, ti, :],
                                         start=(ti == 0), stop=(ti == 2))
                    bcs = spool.tile([128, PC], f32, tag="bcs")
                    nc.scalar.activation(bcs[:], bc[:], AF.Identity)
                    for b in range(2):
                        po = o_ps.tile([128, S], f32, tag="o")
                        for hh in range(2):
                            h = 2 * c + hh
                            for s in range(2):
                                seg = ((h * 2 + b) * 2 + s)
                                nc.tensor.matmul(
                                    po[hh * 64:hh * 64 + 64, :],
                                    vtm[:TK, b * 2 + s, h * 64:(h + 1) * 64],
                                    eT[:TK, seg, :],
                                    start=(s == 0), stop=(s == 1))
                        nc.vector.tensor_tensor(ofm[:, c, b * S:(b + 1) * S],
                                                bcs[:, b * S:(b + 1) * S],
                                                po[:, :], ALU.mult)
                # ---------------- Wo + residual -> y ; LN1 -> X, xb
                y = spool.tile([128, NC_CH, PC], f32, tag="mid8")
                for c in range(NC_CH):
                    ps = mm_ps.tile([128, PC], f32, tag="mm")
                    for c2 in range(NC_CH):
                        nc.tensor.matmul(ps[:], wo[:, c2, c * 128:(c + 1) * 128],
                                         ofm[:, c2, :],
                                         start=(c2 == 0), stop=(c2 == NC_CH - 1))
                    nc.scalar.activation(y[:, c, :], ps[:], AF.Identity,
                                         bias=cv[:, 8 + c:9 + c])
                    nc.vector.tensor_tensor(y[:, c, :], y[:, c, :], X[:, c, pc],
                                            ALU.add)
                _layernorm(nc, tc, spool, tpool, sc_ps, y, X, xb, pc, cv,
                           32, 36, onesbf, onesrbf, rs_ps)
                # ---------------- FFN
                hb_t = spool2.tile([128, NF_CH, PC], bf16, tag="big16")
                for fch in range(NF_CH):
                    ps = mm_ps.tile([128, PC], f32, tag="mm")
                    for c in range(NC_CH):
                        nc.tensor.matmul(ps[:], wf1[:, c, fch * 128:(fch + 1) * 128],
                                         xb[:, c, :],
                                         start=(c == 0), stop=(c == NC_CH - 1))
                    nc.vector.tensor_scalar(hb_t[:, fch, :], ps[:],
                                            cv[:, 12 + fch:13 + fch], 0.0,
                                            ALU.add, ALU.max)
                for c in range(NC_CH):
                    ps = mm_ps.tile([128, PC], f32, tag="mm")
                    for fch in range(NF_CH):
                        nc.tensor.matmul(ps[:], wf2[:, fch, c * 128:(c + 1) * 128],
                                         hb_t[:, fch, :],
                                         start=(fch == 0), stop=(fch == NF_CH - 1))
                    nc.scalar.activation(y[:, c, :], ps[:], AF.Identity,
                                         bias=cv[:, 28 + c:29 + c])
                    nc.vector.tensor_tensor(y[:, c, :], y[:, c, :], X[:, c, pc],
                                            ALU.add)
                _layernorm(nc, tc, spool, tpool, sc_ps, y, X, xb, pc, cv,
                           40, 44, onesbf, onesrbf, rs_ps)

        # -------------------------------------------------- classifier
        psc = rs_ps.tile([C, BPC], f32, tag="rs")
        for c in range(NC_CH):
            nc.tensor.matmul(psc[:], clsw[:, c, :],
                             X[:, c, :].rearrange("p (b t) -> p b t", b=BPC, t=S)
                             [:, :, S - 1],
                             start=(c == 0), stop=(c == NC_CH - 1))
        osb = spool.tile([C, BPC], f32, tag="osb")
        nc.scalar.activation(osb[:], psc[:], AF.Identity, bias=clsb[:, 0:1])
        nc.sync.dma_start(dout[:], osb[:])


def _layernorm(nc, tc, spool, tpool, sc_ps, y, X, xb, pc, cv, gcol, bcol,
               onesbf, onesrbf, rs_ps=None):
    """Feature-major LN over D=512: stats via ones-matmul, broadcasts via
    fp32 rank-1 matmul, apply via DVE + per-partition g/b.
    Writes result fp32 into X[:, :, pc] and bf16 into xb."""
    ybf = spool.tile([128, NC_CH, PC], bf16, tag="ybf")
    ysq = spool.tile([128, NC_CH, PC], bf16, tag="ysq")
    for c in range(NC_CH):
        nc.vector.tensor_copy(ybf[:, c, :], y[:, c, :])
        nc.vector.tensor_tensor(ysq[:, c, :], y[:, c, :], y[:, c, :], ALU.mult)
    lnps = rs_ps if rs_ps is not None else sc_ps
    psA = lnps.tile([1, PC], f32, tag="rs")
    for c in range(NC_CH):
        nc.tensor.matmul(psA[0:1, :], onesbf[:, 0:1], ybf[:, c, :],
                         start=(c == 0), stop=(c == NC_CH - 1))
    psB = lnps.tile([1, PC], f32, tag="rs")
    for c in range(NC_CH):
        nc.tensor.matmul(psB[0:1, :], onesbf[:, 0:1], ysq[:, c, :],
                         start=(c == 0), stop=(c == NC_CH - 1))
    # single-partition scalar math: st segs: 0=mu, 1=var->sd->rstd, 2=tmp
    st = spool.tile([1, 3, PC], f32, tag="st")
    nc.vector.tensor_scalar(st[0:1, 0, :], psA[0:1, :], 1.0 / D, None, ALU.mult)
    nc.vector.tensor_scalar(st[0:1, 1, :], psB[0:1, :], 1.0 / D, None, ALU.mult)
    nc.vector.tensor_tensor(st[0:1, 2, :], st[0:1, 0, :], st[0:1, 0, :], ALU.mult)
    nc.vector.tensor_tensor(st[0:1, 1, :], st[0:1, 1, :], st[0:1, 2, :],
                            ALU.subtract)
    nc.vector.tensor_scalar(st[0:1, 1, :], st[0:1, 1, :], EPS, None, ALU.add)
    nc.scalar.sqrt(st[0:1, 2, :], st[0:1, 1, :])
    stbf = spool.tile([1, 2, PC], bf16, tag="stbf")
    with nc.allow_low_precision(reason="ln rstd"):
        nc.vector.reciprocal(stbf[0:1, 1, :], st[0:1, 2, :])
        nc.vector.tensor_copy(stbf[0:1, 0, :], st[0:1, 0, :])
    # broadcasts: A = rstd, B = mu  ([128, PC] psum, bf16 rank-1 matmul)
    bcA = lnps.tile([128, PC], f32, tag="rs")
    nc.tensor.matmul(bcA[:], onesf[0:1, :].bitcast(bf16)[0:1, 0:128] if False
                     else onesrbf[0:1, :], stbf[0:1, 1, :],
                     start=True, stop=True)
    bcB = lnps.tile([128, PC], f32, tag="rs")
    nc.tensor.matmul(bcB[:], onesrbf[0:1, :], stbf[0:1, 0, :],
                     start=True, stop=True)
    for c in range(NC_CH):
        nc.vector.tensor_tensor(y[:, c, :], y[:, c, :], bcB[:], ALU.subtract)
        nc.vector.tensor_tensor(y[:, c, :], y[:, c, :], bcA[:], ALU.mult)
        nc.vector.tensor_scalar(X[:, c, pc], y[:, c, :],
                                cv[:, gcol + c:gcol + c + 1],
                                cv[:, bcol + c:bcol + c + 1],
                                ALU.mult, ALU.add)
        nc.vector.tensor_copy(xb[:, c, :], X[:, c, pc])


# ---------------------------------------------------------------- host side
_CACHED = {}


def _prep_consts(inputs):
    bf = ml_dtypes.bfloat16
    mask = _mask_np()
    maskbit = (mask == 0.0).astype(np.float32)          # [S(q), S(k)]
    maskT = maskbit.T                                   # [k, q]
    maskb = maskT.reshape(2, TK, S).transpose(1, 0, 2).astype(bf)

    def chunk_pm(vec):                                  # [D] -> [128, 4]
        return np.ascontiguousarray(vec.reshape(NC_CH, 128).T)

    cons = {}
    cons['maskb'] = np.ascontiguousarray(maskb)
    cons['pw1'] = inputs['pW1'].astype(bf)
    cons['pw2'] = np.ascontiguousarray(
        inputs['pW2'].reshape(128, NC_CH, 128).transpose(0, 1, 2)
        .reshape(128, NC_CH * 128)).astype(bf)
    cons['bw1'] = inputs['bW1'].astype(bf)
    cons['bw2'] = np.ascontiguousarray(
        inputs['bW2'].reshape(128, NC_CH, 128).reshape(128, NC_CH * 128)).astype(bf)
    cons['pb1'] = inputs['pb1'].reshape(128, 1).astype(np.float32)
    cons['pb2s'] = (chunk_pm(inputs['pb2']) * SCALE).astype(np.float32)
    cons['bb1'] = inputs['bb1'].reshape(128, 1).astype(np.float32)
    cons['bb2s'] = (chunk_pm(inputs['bb2']) * SCALE).astype(np.float32)

    # weights: lhsT chunk layout [l, 128, cin_chunk, cols]
    def wlay(w, nch):
        # w: [L, K, N] -> [L, 128, nch, N]
        Lw, K, N = w.shape
        return np.ascontiguousarray(
            w.reshape(Lw, nch, 128, N).transpose(0, 2, 1, 3)).astype(bf)

    cons['wqkv'] = wlay(inputs['Wqkv'], NC_CH)
    cons['wo'] = wlay(inputs['Wo'], NC_CH)
    cons['wf1'] = wlay(inputs['Wf1'], NC_CH)
    cons['wf2'] = wlay(inputs['Wf2'], NF_CH)

    cvec = np.zeros((128, L, 48), np.float32)
    for l in range(L):
        bq = inputs['bqkv'][l]
        cvec[:, l, 0:4] = chunk_pm(bq[0:D]) * HSCALE
        cvec[:, l, 4:8] = chunk_pm(bq[D:2 * D])
        bo_eff = inputs['bo'][l] + bq[2 * D:3 * D] @ inputs['Wo'][l]
        cvec[:, l, 8:12] = chunk_pm(bo_eff)
        cvec[:, l, 12:28] = np.ascontiguousarray(
            inputs['bf1'][l].reshape(NF_CH, 128).T)
        cvec[:, l, 28:32] = chunk_pm(inputs['bf2'][l])
        cvec[:, l, 32:36] = chunk_pm(inputs['ln1g'][l])
        cvec[:, l, 36:40] = chunk_pm(inputs['ln1b'][l])
        cvec[:, l, 40:44] = chunk_pm(inputs['ln2g'][l])
        cvec[:, l, 44:48] = chunk_pm(inputs['ln2b'][l])
    cons['cvec'] = cvec

    selcf = np.zeros((96, 3, NC_CH * 128), np.float32)
    for h in range(H):
        ti, sub = divmod(h, 3)
        c, half = divmod(h, 2)
        selcf[sub * 32, ti, c * 128 + half * 64: c * 128 + half * 64 + 64] = 1.0
    cons['selcf'] = selcf.astype(ml_dtypes.bfloat16)
    cons['onesbf'] = np.ones((128, 1), bf)
    cons['onesf'] = np.ones((1, 128), np.float32)
    cons['clsw'] = np.ascontiguousarray(
        inputs['clsW'].reshape(NC_CH, 128, C).transpose(1, 0, 2)).astype(np.float32)
    cons['clsb'] = inputs['clsb'].reshape(C, 1).astype(np.float32)
    cons['clse'] = chunk_pm(inputs['cls_e']).astype(np.float32)
    return cons


def kernel(**inputs):
    inputs = {k: np.asarray(v) for k, v in inputs.items()}
    bf = ml_dtypes.bfloat16
    if 'nc' not in _CACHED:
        _CACHED['nc'] = build()
    nc = _CACHED['nc']
    cons = _prep_consts(inputs)

    # per-core token features
    emb = inputs['emb'].astype(np.float32)
    pe = emb[inputs['player_idxs'].astype(np.int64)]        # [B, T, P, E]
    pin = np.concatenate([pe,
                          inputs['player_xs'][..., None],
                          inputs['player_ys'][..., None]], -1)  # [B,T,P,66]
    ball_e = np.broadcast_to(inputs['ball_e'], (B, T, E))
    bi = np.concatenate([ball_e,
                         inputs['ball_xs'][..., None],
                         inputs['ball_ys'][..., None]], -1)     # [B,T,66]

    in_maps = []
    for core in range(NCORES):
        bs = slice(core * BPC, (core + 1) * BPC)
        m = dict(cons)
        # pinT: [66, BPC*T*P], col = b*200 + t*10 + p
        m['pinT'] = np.ascontiguousarray(
            pin[bs].reshape(BPC * T * P, E + 2).T).astype(bf)
        m['ballT'] = np.ascontiguousarray(
            bi[bs].reshape(BPC * T, E + 2).T).astype(bf)
        in_maps.append(m)

    res = run_bass_kernel_spmd(nc, in_maps, core_ids=list(range(NCORES)))
    outs = [res.results[c]['out'] for c in range(NCORES)]    # [C, BPC] each
    full = np.concatenate([o.T for o in outs], axis=0)       # [B, C]
    return full.astype(np.float32)


if __name__ == "__main__":
    # smoke-build only
    nc = build(n_layers=1, n_pairs=1)
    print("build ok")

